# revision 54
# baseline (speedup 1.0000x reference)
"""Trainium2 Bass kernel for nn_Attention_52407190945839 (channel attention).

Single fused SPMD launch on 8 NeuronCores, data parallel over
(batch, 64-row strips of H).  The whole problem is transport-bound on the
axon tunnel (device exec is ~0.35 ms), so the design minimizes
host<->device bytes and per-call host overhead:

  - x ships as f16 strips (+2 halo rows); raw weights ship tiny and are
    expanded into the fused (1x1 qkv * 3x3 depthwise) conv weights on
    device.
  - Per core: v = conv(x) kept resident in SBUF (f16); q/k computed in
    flipped layout feeding Gram matmuls accumulated in PSUM.
  - Gram partials are AllReduce'd on device (single [[0..7]] group with
    per-batch slots - cheaper to set up than two groups), then the whole
    16x16-per-head attention math (normalize / softmax / relu^2-gelu
    scale-shift) runs on device in f32, producing one combined 128x128
    output matrix W = proj @ blockdiag(attn).
  - y = W @ v is quantized to int8 with per-channel scales on device
    (HW rounds to nearest) to halve the download.

Wire: ~34 MiB up + ~16 MiB down in ONE launch, vs ~278 MiB over two
launches in the naive split.  The runner AOT-compiles the XLA wrapper at
build time, creates the donated output buffers on device (zeros never
cross the wire), disk-caches the walrus BIR->NEFF compile, and pre-warms
the sharded-transfer channels.
"""

import os
from contextlib import ExitStack

import numpy as np

DIM = 128
HEADS = 8
C = DIM // HEADS       # 16
H = W = 256
B = 2
N_CORES = 8
ROWS = H // 4          # 64 rows per core
L_CORE = ROWS * W      # 16384 positions per core
NTILE = L_CORE // 512  # 32 tiles of 512 for v / output streaming
NCHUNK = L_CORE // 128  # 128 chunks of 128 positions for qk/gram

LAST_TIMING = {}

_CACHED = {}


def _build_host_tensors(x, qkv_w, dw_w, proj_w, attca_w, temperature):
    f16 = np.float16
    qkv2 = qkv_w[:, :, 0, 0]                  # [384, 128]
    dw2 = dw_w[:, 0].reshape(3 * DIM, 9)      # [384, 9]
    qkvT = np.ascontiguousarray(qkv2.T).astype(f16)            # [128, 384]
    dwT = np.ascontiguousarray(dw2.T).reshape(1, -1).astype(f16)  # [1, 9*384]
    projT = np.ascontiguousarray(proj_w[:, :, 0, 0].T).astype(f16)  # [128, 128]
    attca2 = attca_w[:, :, 0, 0]              # [32, 16]
    attca_stack = np.tile(attca2.T, (HEADS, 1)).astype(np.float32)  # [128, 32]
    tvec = np.repeat(temperature.reshape(HEADS), C).reshape(DIM, 1).astype(np.float32)
    ident = np.eye(DIM, dtype=np.float32)
    gsels = [np.tile(np.array([[1.0, 0.0]] if core < 4 else [[0.0, 1.0]],
                              np.float32), (DIM, 1)) for core in range(N_CORES)]

    # xg holds all 8 cores' strips pre-concatenated along the partition
    # axis (the layout the sharded device_put wants, no copy at run time)
    xg = np.zeros((N_CORES * DIM, ROWS + 2, W), dtype=f16)
    xins = []
    for core in range(N_CORES):
        b, quad = divmod(core, 4)
        r0 = quad * ROWS
        xin = xg[core * DIM:(core + 1) * DIM]
        lo = max(r0 - 1, 0)
        hi = min(r0 + ROWS + 1, H)
        xin[:, lo - (r0 - 1):hi - (r0 - 1), :] = x[b, :, lo:hi, :].astype(f16)
        xins.append(xin)
    return xins, qkvT, dwT, projT, attca_stack, tvec, ident, gsels, xg


# ---------------------------------------------------------------- device ----

def _build_kernel(bacc, mybir, tile, n_cores=N_CORES):
    nc = bacc.Bacc("TRN2", target_bir_lowering=False, debug=False,
                   num_devices=n_cores)
    f32 = mybir.dt.float32
    f16 = mybir.dt.float16
    alu = mybir.AluOpType
    act = mybir.ActivationFunctionType

    xin = nc.dram_tensor("xin", [DIM, ROWS + 2, W], f16, kind="ExternalInput").ap()
    qkvT = nc.dram_tensor("qkvT", [DIM, 3 * DIM], f16, kind="ExternalInput").ap()
    dwT = nc.dram_tensor("dwT", [1, 9 * 3 * DIM], f16, kind="ExternalInput").ap()
    projT = nc.dram_tensor("projT", [DIM, DIM], f16, kind="ExternalInput").ap()
    attca = nc.dram_tensor("attca", [DIM, 2 * C], f32, kind="ExternalInput").ap()
    tvec = nc.dram_tensor("tvec", [DIM, 1], f32, kind="ExternalInput").ap()
    gsel = nc.dram_tensor("gsel", [DIM, 2], f32, kind="ExternalInput").ap()
    ident = nc.dram_tensor("ident", [DIM, DIM], f32, kind="ExternalInput").ap()
    yout = nc.dram_tensor("yout", [DIM, L_CORE], mybir.dt.int8,
                          kind="ExternalOutput").ap()
    ysc = nc.dram_tensor("ysc", [DIM, 1], f32, kind="ExternalOutput").ap()

    with tile.TileContext(nc) as tc, ExitStack() as ctx:
        const = ctx.enter_context(tc.tile_pool(name="const", bufs=1))
        qkpool = ctx.enter_context(tc.tile_pool(name="qksb", bufs=4))
        opool = ctx.enter_context(tc.tile_pool(name="osb", bufs=4))
        attsb = ctx.enter_context(tc.tile_pool(name="attsb", bufs=1))
        psv = ctx.enter_context(tc.tile_pool(name="psv", bufs=2, space="PSUM"))
        psqk = ctx.enter_context(tc.tile_pool(name="psqk", bufs=2, space="PSUM"))
        psg = ctx.enter_context(tc.tile_pool(name="psg", bufs=1, space="PSUM"))
        psm = ctx.enter_context(tc.tile_pool(name="psm", bufs=1, space="PSUM"))
        dram = ctx.enter_context(tc.tile_pool(name="dram", bufs=1, space="DRAM"))

        # ---- constant loads
        qkvT_sb = const.tile([DIM, 3 * DIM], f16)
        nc.sync.dma_start(qkvT_sb[:], qkvT)
        dwT_sb = const.tile([1, 9 * 3 * DIM], f16)
        nc.sync.dma_start(dwT_sb[:], dwT)
        projT_sb = const.tile([DIM, DIM], f16)
        nc.sync.dma_start(projT_sb[:], projT)
        attca_sb = const.tile([DIM, 2 * C], f32)
        nc.sync.dma_start(attca_sb[:], attca)
        tvec_sb = const.tile([DIM, 1], f32)
        nc.sync.dma_start(tvec_sb[:], tvec)
        gsel_sb = const.tile([DIM, 2], f32)
        nc.sync.dma_start(gsel_sb[:], gsel)
        ident_sb = const.tile([DIM, DIM], f32)
        nc.sync.dma_start(ident_sb[:], ident)

        # x strip, padded in W on device (2 zero columns)
        xsb = const.tile([DIM, ROWS + 2, W + 2], f16)
        nc.vector.memset(xsb[:, :, 0:1], 0.0)
        nc.vector.memset(xsb[:, :, W + 1:W + 2], 0.0)
        row_chunks = [(0, 18), (18, 34), (34, 50), (50, ROWS + 2)]
        for lo, hi in row_chunks:
            nc.sync.dma_start(xsb[:, lo:hi, 1:W + 1], xin[:, lo:hi, :])

        # ---- expand fused conv weights on device:
        #   W2T[i, o, tap] = qkvT[i, o] * dwT[tap, o]
        ones16 = const.tile([1, DIM], f16)
        nc.vector.memset(ones16[:], 1.0)
        wv_sb = const.tile([DIM, 9 * DIM], f16)       # lhsT for v-conv
        wqk_sb = const.tile([DIM, 9 * 2 * DIM], f16)  # rhs for flipped qk-conv
        # one shared PSUM scratch tile (1 bank) for all small matmuls
        patt = psm.tile([DIM, 3 * DIM], f32)
        for tap in range(9):
            dwbc = patt
            nc.tensor.matmul(
                dwbc[:], lhsT=ones16[:],
                rhs=dwT_sb[0:1, tap * 3 * DIM:(tap + 1) * 3 * DIM],
                start=True, stop=True)
            nc.vector.tensor_tensor(
                out=wqk_sb[:, tap * 2 * DIM:(tap + 1) * 2 * DIM],
                in0=qkvT_sb[:, :2 * DIM], in1=dwbc[:, :2 * DIM], op=alu.mult)
            nc.vector.tensor_tensor(
                out=wv_sb[:, tap * DIM:(tap + 1) * DIM],
                in0=qkvT_sb[:, 2 * DIM:], in1=dwbc[:, 2 * DIM:], op=alu.mult)

        # v stays resident in SBUF for the output matmul
        v_all = const.tile([DIM, L_CORE], f16)

        g1t = psg.tile([DIM, 2 * DIM], f32)   # q.q | q.k
        g2t = psg.tile([DIM, DIM], f32)       # k.k
        g1 = g1t[:]
        g2 = g2t[:]

        # ---- main loop: v tiles + qk gram chunks interleaved
        for rp in range(NTILE):
            pv = psv.tile([DIM, 512], f32, tag="vps")
            for tap in range(9):
                dh, dw = divmod(tap, 3)
                nc.tensor.matmul(
                    pv[:],
                    lhsT=wv_sb[:, tap * DIM:(tap + 1) * DIM],
                    rhs=xsb[:, 2 * rp + dh:2 * rp + dh + 2, dw:dw + W],
                    start=(tap == 0), stop=(tap == 8),
                )
            nc.any.tensor_copy(out=v_all[:, rp * 512:(rp + 1) * 512], in_=pv[:])

            for sub in range(4):
                chk = 4 * rp + sub
                r, w0 = divmod(chk, 2)
                w0 *= 128
                pqk = psqk.tile([DIM, 2 * DIM], f32)
                for tap in range(9):
                    dh, dw = divmod(tap, 3)
                    nc.tensor.matmul(
                        pqk[:],
                        lhsT=xsb[:, r + dh, w0 + dw:w0 + dw + 128],
                        rhs=wqk_sb[:, tap * 2 * DIM:(tap + 1) * 2 * DIM],
                        start=(tap == 0), stop=(tap == 8),
                    )
                qkt = qkpool.tile([DIM, 2 * DIM], f16)
                nc.any.tensor_copy(out=qkt[:], in_=pqk[:])
                nc.tensor.matmul(g1, lhsT=qkt[:, :DIM], rhs=qkt[:],
                                 start=(chk == 0), stop=(chk == NCHUNK - 1))
                nc.tensor.matmul(g2, lhsT=qkt[:, DIM:], rhs=qkt[:, DIM:],
                                 start=(chk == 0), stop=(chk == NCHUNK - 1))

        # ---- gram -> AllReduce. A single [[0..7]] group sets up ~0.4s
        # faster than two groups; each batch gets its own 384-col slot,
        # selected by the per-core gsel input ([1,0] batch0, [0,1] batch1).
        gsb = attsb.tile([DIM, 3 * DIM], f32)
        nc.any.tensor_copy(out=gsb[:, :2 * DIM], in_=g1)
        nc.any.tensor_copy(out=gsb[:, 2 * DIM:], in_=g2)
        gcat = attsb.tile([DIM, 6 * DIM], f32)
        nc.vector.tensor_scalar_mul(out=gcat[:, :3 * DIM], in0=gsb[:],
                                    scalar1=gsel_sb[:, 0:1])
        nc.vector.tensor_scalar_mul(out=gcat[:, 3 * DIM:], in0=gsb[:],
                                    scalar1=gsel_sb[:, 1:2])
        gin = dram.tile([DIM, 6 * DIM], f32)
        gout = dram.tile([DIM, 6 * DIM], f32)
        nc.gpsimd.dma_start(gin[:], gcat[:])
        nc.gpsimd.collective_compute(
            "AllReduce", mybir.AluOpType.add,
            replica_groups=[list(range(n_cores))],
            ins=[gin.opt()], outs=[gout.opt()],
        )
        gcat2 = attsb.tile([DIM, 6 * DIM], f32)
        nc.gpsimd.dma_start(gcat2[:], gout[:])
        Gsb0 = attsb.tile([DIM, 3 * DIM], f32)
        nc.vector.tensor_scalar_mul(out=Gsb0[:], in0=gcat2[:, :3 * DIM],
                                    scalar1=gsel_sb[:, 0:1])
        Gsb = attsb.tile([DIM, 3 * DIM], f32)
        nc.vector.scalar_tensor_tensor(
            out=Gsb[:], in0=gcat2[:, 3 * DIM:], scalar=gsel_sb[:, 1:2],
            in1=Gsb0[:], op0=alu.mult, op1=alu.add)

        # ---- attention math (f32, tiny)
        # row norms: dq = diag(G_qq), dk = diag(G_kk)
        tmpq = attsb.tile([DIM, DIM], f32)
        nc.vector.tensor_mul(tmpq[:], Gsb[:, :DIM], ident_sb[:])
        dq = attsb.tile([DIM, 1], f32)
        nc.vector.tensor_reduce(dq[:], tmpq[:], axis=mybir.AxisListType.X, op=alu.add)
        tmpk = attsb.tile([DIM, DIM], f32)
        nc.vector.tensor_mul(tmpk[:], Gsb[:, 2 * DIM:], ident_sb[:])
        dk = attsb.tile([DIM, 1], f32)
        nc.vector.tensor_reduce(dk[:], tmpk[:], axis=mybir.AxisListType.X, op=alu.add)

        nq = attsb.tile([DIM, 1], f32)
        nc.scalar.activation(nq[:], dq[:], act.Sqrt)
        nqc = attsb.tile([DIM, 1], f32)
        nc.vector.tensor_scalar_max(nqc[:], nq[:], 1e-12)
        rq = attsb.tile([DIM, 1], f32)
        nc.vector.reciprocal(rq[:], nqc[:])

        nk = attsb.tile([DIM, 1], f32)
        nc.scalar.activation(nk[:], dk[:], act.Sqrt)
        nkc = attsb.tile([DIM, 1], f32)
        nc.vector.tensor_scalar_max(nkc[:], nk[:], 1e-12)
        rk = attsb.tile([DIM, 1], f32)
        nc.vector.reciprocal(rk[:], nkc[:])

        # broadcast rk along free dim: rkbc[p, d] = rk[d]
        rkrow_ps = patt[0:1, 0:DIM]
        nc.tensor.matmul(rkrow_ps, lhsT=rk[:], rhs=ident_sb[:],
                         start=True, stop=True)
        rkrow = attsb.tile([1, DIM], f32)
        nc.any.tensor_copy(rkrow[:], rkrow_ps)
        onesf = attsb.tile([1, DIM], f32)
        nc.vector.memset(onesf[:], 1.0)
        rkbc_ps = patt[:, DIM:2 * DIM]
        nc.tensor.matmul(rkbc_ps, lhsT=onesf[:], rhs=rkrow[:],
                         start=True, stop=True)

        # A = G_qk * rq[rows] * rk[cols]
        A = attsb.tile([DIM, DIM], f32)
        nc.vector.scalar_tensor_tensor(
            out=A[:], in0=Gsb[:, DIM:2 * DIM], scalar=rq[:, 0:1],
            in1=rkbc_ps, op0=alu.mult, op1=alu.mult)

        # extract per-head diagonal blocks (DMA: engines need 32-aligned
        # partition offsets, DMA does not), then * temperature
        attnraw = attsb.tile([DIM, C], f32)
        for h in range(HEADS):
            nc.sync.dma_start(attnraw[h * C:(h + 1) * C, :],
                              A[h * C:(h + 1) * C, h * C:h * C + C])
        attnb = attsb.tile([DIM, C], f32)
        nc.vector.tensor_scalar_mul(out=attnb[:], in0=attnraw[:],
                                    scalar1=tvec_sb[:, 0:1])

        # softmax over the 16-wide free dim
        rowmax = attsb.tile([DIM, 1], f32)
        nc.vector.tensor_reduce(rowmax[:], attnb[:], axis=mybir.AxisListType.X,
                                op=alu.max)
        attns = attsb.tile([DIM, C], f32)
        nc.vector.tensor_scalar(out=attns[:], in0=attnb[:],
                                scalar1=rowmax[:, 0:1], scalar2=None,
                                op0=alu.subtract)
        attne = attsb.tile([DIM, C], f32)
        rowsum = attsb.tile([DIM, 1], f32)
        nc.scalar.activation(attne[:], attns[:], act.Exp,
                             accum_out=rowsum[:, 0:1])
        rs_r = attsb.tile([DIM, 1], f32)
        nc.vector.reciprocal(rs_r[:], rowsum[:])
        attn0 = attsb.tile([DIM, C], f32)
        nc.vector.tensor_scalar_mul(out=attn0[:], in0=attne[:],
                                    scalar1=rs_r[:, 0:1])

        # a1 = relu(attn)^2 ; a1g = gelu(a1) * a1
        ar = attsb.tile([DIM, C], f32)
        nc.vector.tensor_scalar_max(ar[:], attnb[:], 0.0)
        a1 = attsb.tile([DIM, C], f32)
        nc.scalar.activation(a1[:], ar[:], act.Square)
        # gelu(a1) via tanh approximation (sim lacks Gelu/Erf; abs err
        # ~2e-4 on [0,1], far under the 2e-2 budget)
        asq = attsb.tile([DIM, C], f32)
        nc.scalar.activation(asq[:], a1[:], act.Square)
        z3 = attsb.tile([DIM, C], f32)
        nc.vector.tensor_mul(z3[:], asq[:], a1[:])
        u = attsb.tile([DIM, C], f32)
        nc.vector.scalar_tensor_tensor(out=u[:], in0=z3[:], scalar=0.044715,
                                       in1=a1[:], op0=alu.mult, op1=alu.add)
        th = attsb.tile([DIM, C], f32)
        nc.scalar.activation(th[:], u[:], act.Tanh, scale=0.7978845608028654)
        w1 = attsb.tile([DIM, C], f32)
        nc.vector.tensor_scalar_add(w1[:], th[:], 1.0)
        hg = attsb.tile([DIM, C], f32)
        nc.vector.scalar_tensor_tensor(out=hg[:], in0=a1[:], scalar=0.5,
                                       in1=w1[:], op0=alu.mult, op1=alu.mult)
        a1g = attsb.tile([DIM, C], f32)
        nc.vector.tensor_mul(a1g[:], hg[:], a1[:])

        # scale/shift = blockdiag(a1)^T @ attca_stack
        A1bd = attsb.tile([DIM, DIM], f32)
        nc.vector.memset(A1bd[:], 0.0)
        for h in range(HEADS):
            nc.sync.dma_start(A1bd[h * C:(h + 1) * C, h * C:h * C + C],
                              a1g[h * C:(h + 1) * C, :])
        A1T_ps = patt[:, 2 * DIM:3 * DIM]
        nc.tensor.transpose(A1T_ps, A1bd[:], ident_sb[:])
        A1T = attsb.tile([DIM, DIM], f32)
        nc.any.tensor_copy(A1T[:], A1T_ps)
        ss_ps = patt[:, 0:2 * C]
        nc.tensor.matmul(ss_ps, lhsT=A1T[:], rhs=attca_sb[:],
                         start=True, stop=True)

        # attn_f = attn0 * (1 + scale) + shift
        t1 = attsb.tile([DIM, C], f32)
        nc.vector.tensor_mul(t1[:], attn0[:], patt[:, 0:C])
        t2 = attsb.tile([DIM, C], f32)
        nc.vector.tensor_add(t2[:], t1[:], attn0[:])
        attn_f = attsb.tile([DIM, C], f32)
        nc.vector.tensor_add(attn_f[:], t2[:], patt[:, C:2 * C])

        # W^T = blockdiag(attn_f)^T @ proj^T  (lhsT for the y matmul)
        attn_f16 = attsb.tile([DIM, C], f16)
        nc.any.tensor_copy(attn_f16[:], attn_f[:])
        bd = attsb.tile([DIM, DIM], f16)
        nc.vector.memset(bd[:], 0.0)
        for h in range(HEADS):
            nc.sync.dma_start(bd[h * C:(h + 1) * C, h * C:h * C + C],
                              attn_f16[h * C:(h + 1) * C, :])
        wc_ps = patt[:, DIM:2 * DIM]
        nc.tensor.matmul(wc_ps, lhsT=bd[:], rhs=projT_sb[:],
                         start=True, stop=True)
        wcl = attsb.tile([DIM, DIM], f16)
        nc.any.tensor_copy(wcl[:], wc_ps)

        # ---- y = W @ v, int8 with per-channel scale (download is the
        # wire bottleneck; int8 halves it, HW converts round-to-nearest)
        # pass 1: per-channel abs-max of y
        ymaxs = attsb.tile([DIM, NTILE], f32)
        for t in range(NTILE):
            py = psv.tile([DIM, 512], f32, tag="vps")
            nc.tensor.matmul(py[:], lhsT=wcl[:], rhs=v_all[:, t * 512:(t + 1) * 512],
                             start=True, stop=True)
            nc.vector.tensor_reduce(ymaxs[:, t:t + 1], py[:],
                                    axis=mybir.AxisListType.X, op=alu.max,
                                    apply_absolute_value=True)
        ymax = attsb.tile([DIM, 1], f32)
        nc.vector.tensor_reduce(ymax[:], ymaxs[:], axis=mybir.AxisListType.X,
                                op=alu.max)
        ymc = attsb.tile([DIM, 1], f32)
        nc.vector.tensor_scalar_max(ymc[:], ymax[:], 1e-20)
        ysc_sb = attsb.tile([DIM, 1], f32)
        nc.vector.tensor_scalar_mul(ysc_sb[:], ymc[:], 1.0 / 127.0)
        nc.sync.dma_start(ysc, ysc_sb[:])
        rs_y = attsb.tile([DIM, 1], f32)
        nc.vector.reciprocal(rs_y[:], ysc_sb[:])
        # pass 2: recompute tiles, quantize straight from PSUM
        for t in range(NTILE):
            py = psv.tile([DIM, 512], f32, tag="vps")
            nc.tensor.matmul(py[:], lhsT=wcl[:], rhs=v_all[:, t * 512:(t + 1) * 512],
                             start=True, stop=True)
            ysb = opool.tile([DIM, 512], mybir.dt.int8)
            nc.vector.tensor_scalar_mul(out=ysb[:], in0=py[:],
                                        scalar1=rs_y[:, 0:1])
            nc.sync.dma_start(yout[:, t * 512:(t + 1) * 512], ysb[:])
    nc.compile()
    return nc


def _install_neff_cache():
    """Disk-cache the walrus BIR->NEFF compile (keyed by BIR hash).

    The compile is deterministic, ~0.25s, and re-runs on every call
    because the jit closure is rebuilt each time; a content-addressed
    cache removes it from the hot path."""
    import hashlib
    import shutil

    import concourse.bass2jax as b2j

    if getattr(b2j, "_ant_neff_cache_installed", False):
        return
    orig = b2j.compile_bir_kernel
    cache_dir = "/tmp/bass_neff_cache"

    def cached(bir_json, tmpdir, neff_name="file.neff"):
        try:
            os.makedirs(cache_dir, exist_ok=True)
            key = hashlib.sha256(bir_json).hexdigest()[:32]
            cpath = os.path.join(cache_dir, f"{key}_{neff_name}")
            if os.path.exists(cpath):
                dst = os.path.join(tmpdir, neff_name)
                shutil.copy(cpath, dst)
                return dst
        except Exception:
            cpath = None
        out = orig(bir_json, tmpdir, neff_name=neff_name)
        if cpath:
            try:
                shutil.copy(out, cpath)
            except Exception:
                pass
        return out

    b2j.compile_bir_kernel = cached
    b2j._ant_neff_cache_installed = True


def _make_runner(nc, device_slice=None):
    """AOT-compiled PJRT runner (replaces run_bass_via_pjrt's hot path).

    - the XLA/NEFF compile happens here (build phase), not per call;
    - donated output buffers are created on device (zeros are not
      shipped over the slow axon tunnel);
    - device_slice picks a subset of jax.devices() (per-batch launches
      on disjoint 4-core submeshes can overlap their transfers)."""
    import jax
    import jax.numpy as jnp
    from jax.sharding import Mesh, NamedSharding, PartitionSpec

    try:
        # keep HLO hashes (and so the on-disk NEFF cache keys) independent
        # of the directory kernel.py is imported from
        jax.config.update("jax_hlo_source_file_canonicalization_regex", ".*")
    except Exception:
        pass

    from jax.experimental.shard_map import shard_map

    import concourse.mybir as mybir
    from concourse import bass2jax

    bass2jax.install_neuronx_cc_hook()
    assert nc.dbg_addr is None
    partition_name = (nc.partition_id_tensor.name
                      if nc.partition_id_tensor else None)

    in_names, in_shapes = [], []
    out_names, out_avals = [], []
    for alloc in nc.m.functions[0].allocations:
        if not isinstance(alloc, mybir.MemoryLocationSet):
            continue
        name = alloc.memorylocations[0].name
        if alloc.kind == "ExternalInput":
            if name != partition_name:
                in_names.append(name)
                in_shapes.append((tuple(alloc.tensor_shape),
                                  mybir.dt.np(alloc.dtype)))
        elif alloc.kind == "ExternalOutput":
            out_names.append(name)
            out_avals.append(jax.core.ShapedArray(
                tuple(alloc.tensor_shape), mybir.dt.np(alloc.dtype)))
    n_params, n_outs = len(in_names), len(out_names)
    bind_names = in_names + out_names
    if partition_name is not None:
        bind_names = bind_names + [partition_name]
    bind_names = tuple(bind_names)
    donate = tuple(range(n_params, n_params + n_outs))

    def _body(*args):
        operands = list(args)
        if partition_name is not None:
            operands.append(bass2jax.partition_id_tensor())
        outs = bass2jax._bass_exec_p.bind(
            *operands, out_avals=tuple(out_avals), in_names=bind_names,
            out_names=tuple(out_names), lowering_input_output_aliases=(),
            sim_require_finite=True, sim_require_nnan=True, nc=nc)
        return tuple(outs)

    devices = (jax.devices()[:N_CORES] if device_slice is None
               else [jax.devices()[i] for i in device_slice])
    n_cores = len(devices)
    mesh = Mesh(np.asarray(devices), ("core",))
    P = PartitionSpec
    sharded = jax.jit(
        shard_map(_body, mesh=mesh, in_specs=(P("core"),) * (n_params + n_outs),
                  out_specs=(P("core"),) * n_outs, check_rep=False),
        donate_argnums=donate, keep_unused=True)
    sh = NamedSharding(mesh, P("core"))

    def _zeros():
        return tuple(jnp.zeros((n_cores * a.shape[0], *a.shape[1:]), a.dtype)
                     for a in out_avals)

    zjit = jax.jit(_zeros, out_shardings=(sh,) * n_outs)

    structs = [jax.ShapeDtypeStruct((n_cores * s[0], *s[1:]), d, sharding=sh)
               for (s, d) in in_shapes]
    structs += [jax.ShapeDtypeStruct((n_cores * a.shape[0], *a.shape[1:]),
                                     a.dtype, sharding=sh) for a in out_avals]
    compiled = sharded.lower(*structs).compile()
    zjit()  # compile the on-device zeros fn now too
    # first sharded device_put in a process pays ~3s of channel setup;
    # absorb it here with tiny transfers
    for wdt in (np.float16, np.float32):
        jax.device_put(np.zeros((n_cores, 2), wdt), sh).block_until_ready()

    def run(in_globals):
        args = [in_globals[nm] for nm in in_names]
        zeros = zjit()
        out_arrs = compiled(*args, *zeros)
        return [{nm: np.asarray(out_arrs[i]).reshape(
                    n_cores, *out_avals[i].shape)[c]
                 for i, nm in enumerate(out_names)}
                for c in range(n_cores)]

    return run


def _get_nc():
    if "ready" not in _CACHED:
        import concourse.bacc as bacc
        import concourse.mybir as mybir
        import concourse.tile as tile
        _install_neff_cache()
        import jax
        jax.devices()  # bring up the PJRT/axon client outside the run window
        # NOTE: per-batch 4-core split launches were tried and rejected:
        # collective NEFFs only load on a device set starting at core 0
        # (LoadExecutable fails on a 4-7 submesh), and transfers barely
        # overlap within one client connection anyway.
        _CACHED["runner4"] = None
        _CACHED["nc"] = _build_kernel(bacc, mybir, tile)
        try:
            _CACHED["runner"] = _make_runner(_CACHED["nc"])
        except Exception:
            _CACHED["runner"] = None  # fall back to run_bass_kernel_spmd
        _CACHED["ready"] = True
    return _CACHED["nc"]


def _run_device(xins, qkvT, dwT, projT, attca_stack, tvec, ident, gsels, xg):
    from concourse import bass_utils
    import time as _time

    t0 = _time.perf_counter()
    nc = _get_nc()
    t1 = _time.perf_counter()
    core_ids = list(range(N_CORES))
    trace = bool(int(os.environ.get("KERNEL_TRACE", "0")))
    runner4 = _CACHED.get("runner4")
    runner = _CACHED.get("runner")
    if runner4 is not None and not trace:
        half = 4 * DIM
        wtile = {"qkvT": np.tile(qkvT, (4, 1)), "dwT": np.tile(dwT, (4, 1)),
                 "projT": np.tile(projT, (4, 1)),
                 "attca": np.tile(attca_stack, (4, 1)),
                 "tvec": np.tile(tvec, (4, 1)), "ident": np.tile(ident, (4, 1))}
        in_globals_ab = [
            dict(wtile, xin=xg[:half],
                 gsel=np.concatenate(gsels[:4], axis=0)),
            dict(wtile, xin=xg[half:],
                 gsel=np.concatenate(gsels[4:], axis=0)),
        ]
    elif runner is not None and not trace:
        in_globals = {"xin": xg,
                      "qkvT": np.tile(qkvT, (N_CORES, 1)),
                      "dwT": np.tile(dwT, (N_CORES, 1)),
                      "projT": np.tile(projT, (N_CORES, 1)),
                      "attca": np.tile(attca_stack, (N_CORES, 1)),
                      "tvec": np.tile(tvec, (N_CORES, 1)),
                      "ident": np.tile(ident, (N_CORES, 1)),
                      "gsel": np.concatenate(gsels, axis=0)}
    t2 = _time.perf_counter()
    if runner4 is not None and not trace:
        from concurrent.futures import ThreadPoolExecutor
        with ThreadPoolExecutor(2) as ex:
            futs = [ex.submit(runner4[i], in_globals_ab[i]) for i in (0, 1)]
            results = futs[0].result() + futs[1].result()
        exec_ns = None
    elif runner is not None and not trace:
        results = runner(in_globals)
        exec_ns = None
    else:
        if getattr(nc, "num_devices", N_CORES) != N_CORES:
            import concourse.bacc as bacc
            import concourse.mybir as mybir
            import concourse.tile as tile
            nc = _CACHED.setdefault("nc8", _build_kernel(bacc, mybir, tile))
        in_maps = [{"xin": xins[c], "qkvT": qkvT, "dwT": dwT, "projT": projT,
                    "attca": attca_stack, "tvec": tvec, "ident": ident,
                    "gsel": gsels[c]}
                   for c in core_ids]
        res = bass_utils.run_bass_kernel_spmd(nc, in_maps, core_ids, trace=trace)
        results = res.results
        exec_ns = res.exec_time_ns
    t3 = _time.perf_counter()
    LAST_TIMING["kernel_a_ns"] = exec_ns
    LAST_TIMING["build_a_s"] = t1 - t0
    LAST_TIMING["run_a_s"] = t3 - t2
    return [(r["yout"], r["ysc"]) for r in results]


def kernel(x, qkv_w, dw_w, proj_w, attca_w, temperature):
    x = np.asarray(x, dtype=np.float32)
    qkv_w = np.asarray(qkv_w, dtype=np.float32)
    dw_w = np.asarray(dw_w, dtype=np.float32)
    proj_w = np.asarray(proj_w, dtype=np.float32)
    attca_w = np.asarray(attca_w, dtype=np.float32)
    temperature = np.asarray(temperature, dtype=np.float32)

    host = _build_host_tensors(x, qkv_w, dw_w, proj_w, attca_w, temperature)
    youts = _run_device(*host)

    out = np.empty((B, DIM, H, W), dtype=np.float32)
    for core in range(N_CORES):
        b, quad = divmod(core, 4)
        r0 = quad * ROWS
        yq, s = youts[core]
        y = yq.astype(np.float32) * s.reshape(DIM, 1)
        out[b, :, r0:r0 + ROWS, :] = y.reshape(DIM, ROWS, W)
    return out
